# revision 48
# baseline (speedup 1.0000x reference)
"""BiDAF attention Trainium2 kernel (fp16-staged, DMA-roofline oriented).

Full-input contract: kernel(**inputs) takes the unsharded tensors
(context [16,2048,128], query [16,128,128], W [384],
context_mask [16,2048] i32, query_mask [16,128] i32) and returns
G = [16, 2048, 512] f32.

Sharding: data-parallel over batch across 8 NeuronCores (2 batches/core).
Each batch's math is fully local to a core, so no collectives.

Per-batch structure (16 c-tiles of 128 rows, in 4 groups of 4):
  per group (single fused pass):
    PE-transpose ctx (f32) -> ctxT fp16,
    S' = ctxT^T @ (qT*w_cq + w_c)   [s_c folded into the moving operand]
      + rank-1 (s_q + qmask) row    -> row-max m = max_q S' (q2c logits),
    T^T = qTwc^T @ ctxT, eT = exp(T^T + (s_q+qmask) bias),
    e_b = exp(m)*cmask, q2c/z partial accumulation,
    c2q = eT^T @ [q|1] -> divide by in-bank Z col (G2), G3 = ctx*c2q.
  per batch tail: q2c row * 1/z broadcast, then per group G4 = ctx*q2c
    and ONE fp16 store of the full 512-col slab.

The softmax over q is invariant to the per-c s_c term, so including it in
eT is exact; exp() is computed without max-subtraction (|S'| <= ~9, fp16
max 65504). All matmul operands are fp16 or f32-pairs; G is written to
DRAM as fp16 (halves store bytes; adds ~5e-4 rel err against a 2e-2
budget) and upcast to f32 on the host.

Scheduling notes (TimelineSim-verified):
- A PSUM tile may have consumers on only ONE engine: cross-engine readers
  of the same PSUM tile are serialized by the dependency tracker. The c2q
  normalize therefore divides by the in-bank Z column on a single engine
  per PSUM pair-bank (no shared reciprocal tile).
- PSUM pools need bufs=2 so group g+1's producers don't wait on group g's
  consumer (ping-pong serialization).
- Engine roles: Act = spine (ctxT copy, exp); Pool = bulk (G1 casts,
  normalize); DVE = row-max, G3/G4, small glue; PE = matmuls.
"""

import sys

sys.path.insert(0, "/opt/trn_rl_repo")

import numpy as np

import concourse.bass as bass
import concourse.tile as tile
from concourse import mybir
from concourse.masks import make_identity
from concourse.vector_clock import ScopedClock

B, C_LEN, Q_LEN, H = 16, 2048, 128, 128
N_CORES = 8
B_PER_CORE = B // N_CORES          # 2
NT = C_LEN // 128                  # 16 c-tiles per batch
NG = 4                             # groups per batch
GS = NT // NG                      # tiles per group (4)
F32 = mybir.dt.float32
F16 = mybir.dt.float16
I32 = mybir.dt.int32
AX = mybir.AxisListType.X
EXP = mybir.ActivationFunctionType.Exp

MAX_WAITS_PER_INST = 1


def _split_excess_waits(nc, insts):
    """Hoist all but one sync wait per instruction onto preceding nops.

    The walrus build in this container rejects >1 sync wait on an
    instruction's descriptor, while Tile's sem assignment freely attaches
    several. A nop on the same engine right before the instruction stalls
    the engine identically.
    """
    out = []
    for inst in insts:
        si = getattr(inst, "sync_info", None)
        waits = list(si.on_wait) if si is not None and si.on_wait else []
        if len(waits) > MAX_WAITS_PER_INST and type(inst).__name__.startswith("Inst"):
            keep = waits[:MAX_WAITS_PER_INST]
            extra = waits[MAX_WAITS_PER_INST:]
            for i in range(0, len(extra), MAX_WAITS_PER_INST):
                out.append(
                    mybir.InstNoOp(
                        name=nc.get_next_instruction_name(),
                        sync_info=mybir.SyncInfo(
                            on_wait=extra[i : i + MAX_WAITS_PER_INST], on_update=[]
                        ),
                        bass_nofuse=True,
                        engine=inst.engine,
                    )
                )
            inst.sync_info = mybir.SyncInfo(
                on_wait=keep, on_update=list(si.on_update or [])
            )
        out.append(inst)
    return out


class SplitDrainTileContext(tile.TileContext):
    """TileContext whose tail drain splits its sem waits across SP nops.

    The walrus build in this container rejects more than one sync wait on a
    TPB_CTRL instruction; the stock tail drain carries one wait per live proc.
    """

    def _lower_ordered_insts(self, ordered):
        for bb_name in list(ordered.keys()):
            ordered[bb_name] = _split_excess_waits(self.nc, ordered[bb_name])
        return super()._lower_ordered_insts(ordered)

    def _drain_and_barrier(self, tick_clock, wait_clock):
        nc = self.nc
        drain_inst = nc.sync.drain()
        wait_clock.add_sem_waits(
            drain_inst.ins, ScopedClock({None: tick_clock.global_clock})
        )
        si = drain_inst.ins.sync_info
        waits = list(si.on_wait) if si is not None and si.on_wait else []
        if waits:
            drain_inst.ins.sync_info = mybir.SyncInfo(
                on_wait=[], on_update=list(si.on_update or [])
            )
            # spread the tail waits across all engines; the all-engine
            # barrier below joins them, so they complete in parallel
            engs = [nc.sync, nc.vector, nc.scalar, nc.tensor, nc.gpsimd]
            for j, i in enumerate(range(0, len(waits), MAX_WAITS_PER_INST)):
                nop = engs[j % len(engs)].nop()
                nop.ins.sync_info = mybir.SyncInfo(
                    on_wait=waits[i : i + MAX_WAITS_PER_INST], on_update=[]
                )
        nc.all_engine_barrier()
        assert self.sems is not None
        popped = nc._tile_sem_poison_stack.pop()
        assert popped is self._sem_poison
        nc.clear_and_free_semaphores(list(self.sems.allocated().values()))
        nc.all_engine_barrier()


def build_nc() -> bass.Bass:
    nc = bass.Bass()
    ctx_d = nc.dram_tensor("context", [B_PER_CORE, C_LEN, H], F32, kind="ExternalInput")
    qry_d = nc.dram_tensor("query", [B_PER_CORE, Q_LEN, H], F32, kind="ExternalInput")
    w_d = nc.dram_tensor("W", [3 * H], F32, kind="ExternalInput")
    cm_d = nc.dram_tensor("context_mask", [B_PER_CORE, C_LEN], I32, kind="ExternalInput")
    qm_d = nc.dram_tensor("query_mask", [B_PER_CORE, Q_LEN], I32, kind="ExternalInput")
    g_d = nc.dram_tensor("G", [B_PER_CORE, C_LEN, 4 * H], F16, kind="ExternalOutput")

    from contextlib import ExitStack

    # psm bank region columns (f32): q2c partials 0:4, s_q 4:5,
    # z row [0:1,5:21], srowT [0:1,22:86]->f16[1,128],
    # q2crT [0:1,86:150]->f16[1,128], cmT 150:166
    QP0, SQ0, Z0, SRT0, QRT0, CMT0 = 0, 4, 5, 22, 86, 150

    with SplitDrainTileContext(nc) as tc, ExitStack() as es:
        consts = es.enter_context(tc.tile_pool(name="consts", bufs=1))
        bp = es.enter_context(tc.tile_pool(name="bp", bufs=2))
        pmm = es.enter_context(tc.tile_pool(name="pmm", bufs=2, space="PSUM"))
        pct = es.enter_context(tc.tile_pool(name="pct", bufs=2, space="PSUM"))
        pcq = es.enter_context(tc.tile_pool(name="pcq", bufs=2, space="PSUM"))
        psm = es.enter_context(tc.tile_pool(name="psm", bufs=2, space="PSUM"))

        ident16 = consts.tile([128, 128], F16)
        make_identity(nc, ident16)
        ident32 = consts.tile([128, 128], F32)
        make_identity(nc, ident32)
        ones_row16 = consts.tile([1, 128], F16)
        nc.vector.memset(ones_row16, 1.0)
        ones_col16 = consts.tile([128, 1], F16)
        nc.vector.memset(ones_col16, 1.0)
        w_all = consts.tile([128, 3], F32)
        w_q16 = consts.tile([128, 1], F16)

        def emit_loads(b, first=False):
            L = {}
            qry_f32 = bp.tile([128, 128], F32, tag="qry_f32")
            nc.sync.dma_start(out=qry_f32, in_=qry_d[b])
            if first:
                nc.sync.dma_start(out=w_all, in_=w_d.rearrange("(w h) -> h w", w=3))
                nc.vector.tensor_copy(out=w_q16, in_=w_all[:, 1:2])
            ctx_f32 = bp.tile([128, NT, 128], F32, tag="ctx_f32")
            ctx_view = ctx_d[b].rearrange("(t p) h -> p t h", p=128)
            nc.sync.dma_start(out=ctx_f32[:, 0:GS, :], in_=ctx_view[:, 0:GS, :])
            qm_i32 = bp.tile([128, 1], I32, tag="qm_i32")
            nc.sync.dma_start(out=qm_i32, in_=qm_d[b].rearrange("(q o) -> q o", o=1))
            for g in range(1, NG):
                sl = slice(g * GS, (g + 1) * GS)
                nc.sync.dma_start(out=ctx_f32[:, sl, :], in_=ctx_view[:, sl, :])
            cm_i32 = bp.tile([16, 128], I32, tag="cm_i32")
            nc.sync.dma_start(out=cm_i32, in_=cm_d[b].rearrange("(t p) -> t p", p=128))
            L.update(qry_f32=qry_f32, qm_i32=qm_i32, cm_i32=cm_i32, ctx_f32=ctx_f32)
            return L

        def emit_prelim(b, L):
            # query -> fp16 [q|1] moving operand; queryT; qTwc = qT*w_cq+w_c
            qrhs129 = bp.tile([128, 129], F16, tag="qrhs129")
            nc.vector.tensor_copy(out=qrhs129[:, 0:128], in_=L["qry_f32"])
            nc.gpsimd.memset(qrhs129[:, 128:129], 1.0)

            ps_qt = pct.tile([128, 128], F16, tag="ct")
            nc.tensor.transpose(ps_qt, qrhs129[:, 0:128], ident16)
            qT_sb = bp.tile([128, 128], F16, tag="qT_sb")
            nc.vector.tensor_copy(out=qT_sb, in_=ps_qt)
            qTwc = bp.tile([128, 128], F16, tag="qTwc")
            nc.vector.tensor_scalar(
                out=qTwc, in0=qT_sb, scalar1=w_all[:, 2:3], scalar2=w_all[:, 0:1],
                op0=mybir.AluOpType.mult, op1=mybir.AluOpType.add,
            )

            bs = psm.tile([128, 166], F32, tag="bs")
            # s_q column [q,1]
            nc.tensor.matmul(bs[:, SQ0 : SQ0 + 1], qT_sb, w_q16, start=True, stop=True)
            qm_f32 = bp.tile([128, 1], F32, tag="qm_f32")
            nc.vector.tensor_copy(out=qm_f32, in_=L["qm_i32"])
            nc.vector.tensor_scalar(
                out=qm_f32, in0=qm_f32, scalar1=1.0, scalar2=60000.0,
                op0=mybir.AluOpType.subtract, op1=mybir.AluOpType.mult,
            )
            sqm_col = bp.tile([128, 1], F32, tag="sqm_col")
            nc.vector.tensor_add(out=sqm_col, in0=bs[:, SQ0 : SQ0 + 1], in1=qm_f32)
            sqm16 = bp.tile([128, 1], F16, tag="sqm16")
            nc.vector.tensor_copy(out=sqm16, in_=sqm_col)
            nc.tensor.transpose(
                bs[0:1, SRT0 : SRT0 + 64].bitcast(F16), sqm16, ident16
            )
            # (s_q + qmask) row tiled 4x for the rank-1 accumulates
            srow512 = bp.tile([1, 512], F16, tag="srow512")
            srow_src = (
                bs[0:1, SRT0 : SRT0 + 64]
                .bitcast(F16)
                .rearrange("o (r q) -> o r q", r=1)
                .broadcast_to([1, GS, 128])
            )
            nc.vector.tensor_copy(
                out=srow512.rearrange("o (r q) -> o r q", q=128), in_=srow_src
            )

            # context mask: [16,128] i32 -> f32 -> PE transpose -> [c,16] fp16
            cm_f32 = bp.tile([16, 128], F32, tag="cm_f32")
            nc.gpsimd.tensor_copy(out=cm_f32, in_=L["cm_i32"])
            nc.tensor.transpose(
                bs[:, CMT0 : CMT0 + 16], cm_f32, ident32[0:16, 0:16]
            )
            cmf16 = bp.tile([128, 16], F16, tag="cmf16")
            nc.vector.tensor_copy(out=cmf16, in_=bs[:, CMT0 : CMT0 + 16])

            ctxT_buf = bp.tile([128, C_LEN], F16, tag="ctxT_buf")
            eT_buf = bp.tile([128, C_LEN], F16, tag="eT_buf")
            gbuf16 = bp.tile([128, NT, 4 * H], F16, tag="gbuf16")
            m_buf = bp.tile([128, NT], F32, tag="m_buf")
            e_raw = bp.tile([128, NT], F16, tag="e_raw")
            e_b = bp.tile([128, NT], F16, tag="e_b")
            g_view = g_d[b].rearrange("(t p) f -> p t f", p=128)
            L.update(
                qrhs129=qrhs129, qTwc=qTwc, sqm_col=sqm_col, srow512=srow512,
                cmf16=cmf16, ctxT_buf=ctxT_buf, eT_buf=eT_buf, gbuf16=gbuf16,
                m_buf=m_buf, e_raw=e_raw, e_b=e_b,
                g_view=g_view, bs=bs,
            )

        def emit_cast(b, L, half):
            # G1 = fp16(ctx), half a batch per op (fixed overheads amortize);
            # feeds only G3/G4/stores, all far from the spine.
            sl = slice(half * NT // 2, (half + 1) * NT // 2)
            nc.gpsimd.tensor_copy(
                out=L["gbuf16"][:, sl, 0:128], in_=L["ctx_f32"][:, sl, :]
            )

        def emit_spine(b, L, g):
            gsl = slice(g * GS, (g + 1) * GS)
            ctxT_buf, eT_buf = L["ctxT_buf"], L["eT_buf"]
            qTwc, srow512, bs = L["qTwc"], L["srow512"], L["bs"]
            c0 = g * GS * 128

            # ---- spine: transposes -> ctxT -> S'/max -> T^T -> exp ----
            ps_ct = pct.tile([128, 512], F32, tag="ct")
            for j in range(GS):
                t = g * GS + j
                nc.tensor.transpose(
                    ps_ct[:, j * 128 : (j + 1) * 128], L["ctx_f32"][:, t, :], ident32
                )
            nc.scalar.copy(out=ctxT_buf[:, c0 : c0 + 512], in_=ps_ct)

            ps_s = pmm.tile([128, GS, 128], F32, tag="mm")
            for j in range(GS):
                t = g * GS + j
                ctxT = ctxT_buf[:, t * 128 : (t + 1) * 128]
                nc.tensor.matmul(ps_s[:, j, :], ctxT, qTwc, start=True, stop=False)
                nc.tensor.matmul(
                    ps_s[:, j, :], ones_row16, srow512[:, j * 128 : (j + 1) * 128],
                    start=False, stop=True,
                )
            nc.vector.reduce_max(out=L["m_buf"][:, gsl], in_=ps_s, axis=AX)

            ps_tt = pmm.tile([128, 512], F32, tag="mm")
            nc.tensor.matmul(
                ps_tt, qTwc, ctxT_buf[:, c0 : c0 + 512], start=True, stop=True
            )
            nc.scalar.activation(
                out=eT_buf[:, c0 : c0 + 512], in_=ps_tt, func=EXP, bias=L["sqm_col"]
            )

        def emit_ebp(b, L, g):
            # q2c partial accumulation for group g (tiny ops); emitted one
            # group late so PE never stalls on the max->exp->e_b chain.
            gsl = slice(g * GS, (g + 1) * GS)
            bs = L["bs"]
            nc.scalar.activation(
                out=L["e_raw"][:, gsl], in_=L["m_buf"][:, gsl], func=EXP
            )
            nc.vector.tensor_mul(
                out=L["e_b"][:, gsl], in0=L["e_raw"][:, gsl], in1=L["cmf16"][:, gsl]
            )
            nc.tensor.matmul(
                bs[0:1, Z0 + g * GS : Z0 + (g + 1) * GS],
                ones_col16, L["e_b"][:, gsl], start=True, stop=True,
            )
            for j in range(GS):
                t = g * GS + j
                nc.tensor.matmul(
                    bs[:, QP0 + g : QP0 + g + 1],
                    L["gbuf16"][:, t, 0:128],
                    L["e_b"][:, t : t + 1],
                    start=(j == 0),
                    stop=(j == GS - 1),
                )

        def emit_gtail(b, L, g):
            gsl = slice(g * GS, (g + 1) * GS)
            gbuf16, eT_buf, bs = L["gbuf16"], L["eT_buf"], L["bs"]
            # ---- c2q + normalize (G2) + G3, inline per group ----
            pcqs = []
            for j in range(2):
                t0 = g * GS + 2 * j
                ps_cq = pcq.tile([128, 258], F32, tag="cq")
                nc.tensor.matmul(
                    ps_cq[:, 0:129],
                    eT_buf[:, t0 * 128 : (t0 + 1) * 128],
                    L["qrhs129"],
                    start=True, stop=True,
                )
                nc.tensor.matmul(
                    ps_cq[:, 129:258],
                    eT_buf[:, (t0 + 1) * 128 : (t0 + 2) * 128],
                    L["qrhs129"],
                    start=True, stop=True,
                )
                pcqs.append(ps_cq)
            for j in range(2):
                t0 = g * GS + 2 * j
                ps_cq = pcqs[j]
                dr = bp.tile([128, 2], F32, tag=f"dr{t0 % 4}")
                nc.vector.reciprocal(out=dr, in_=ps_cq[:, 128:258:129])
                if b == 0 or j == 0:
                    num = ps_cq.rearrange("p (t c) -> p t c", t=2)[:, :, 0:128]
                    drb = (
                        dr.rearrange("p (t o) -> p t o", o=1)
                        .broadcast_to([128, 2, 128])
                    )
                    nc.vector.tensor_tensor(
                        out=gbuf16[:, t0 : t0 + 2, 128:256], in0=num, in1=drb,
                        op=mybir.AluOpType.mult,
                    )
                else:
                    nc.scalar.mul(
                        out=gbuf16[:, t0, 128:256], in_=ps_cq[:, 0:128],
                        mul=dr[:, 0:1],
                    )
                    nc.scalar.mul(
                        out=gbuf16[:, t0 + 1, 128:256], in_=ps_cq[:, 129:257],
                        mul=dr[:, 1:2],
                    )
            # store cols 0:256 now (512B contiguous rows, full DMA rate);
            # G3+G4 go out together in the batch tail, also at full rate.
            nc.sync.dma_start(
                out=L["g_view"][:, gsl, 0:256], in_=gbuf16[:, gsl, 0:256]
            )
            g3eng = nc.gpsimd if b == 0 else nc.vector
            g3eng.tensor_mul(
                out=gbuf16[:, gsl, 256:384],
                in0=gbuf16[:, gsl, 0:128],
                in1=gbuf16[:, gsl, 128:256],
            )

        def emit_ebtail(b, L):
            bs = L["bs"]
            q2cs = bp.tile([128, 1], F32, tag="q2cs")
            nc.vector.reduce_sum(out=q2cs, in_=bs[:, QP0 : QP0 + NG], axis=AX)
            q2c16 = bp.tile([128, 1], F16, tag="q2c16")
            nc.vector.tensor_copy(out=q2c16, in_=q2cs)
            z_tot = bp.tile([1, 1], F32, tag="z_tot")
            nc.vector.reduce_sum(out=z_tot, in_=bs[0:1, Z0 : Z0 + NT], axis=AX)
            zr = bp.tile([1, 1], F32, tag="zr")
            nc.vector.reciprocal(out=zr, in_=z_tot)
            nc.tensor.transpose(
                bs[0:1, QRT0 : QRT0 + 64].bitcast(F16), q2c16, ident16
            )
            q2cr_sb = bp.tile([1, 128], F16, tag="q2cr_sb")
            nc.vector.tensor_scalar_mul(
                q2cr_sb, bs[0:1, QRT0 : QRT0 + 64].bitcast(F16), zr
            )
            ps_bc = pcq.tile([128, 128], F32, tag="cq")
            nc.tensor.matmul(ps_bc, ones_row16, q2cr_sb, start=True, stop=True)
            bc_sb = bp.tile([128, 128], F16, tag="bc_sb")
            nc.vector.tensor_copy(out=bc_sb, in_=ps_bc)
            L.update(bc_sb=bc_sb)

        def emit_finish(b, L, g):
            gsl = slice(g * GS, (g + 1) * GS)
            gbuf16 = L["gbuf16"]
            bc_bcast = (
                L["bc_sb"].rearrange("p (o q) -> p o q", o=1).broadcast_to([128, GS, 128])
            )
            eng = nc.gpsimd if b == 0 else nc.vector
            eng.tensor_mul(
                out=gbuf16[:, gsl, 384:512],
                in0=gbuf16[:, gsl, 0:128],
                in1=bc_bcast,
            )
            nc.sync.dma_start(
                out=L["g_view"][:, gsl, 256:512], in_=gbuf16[:, gsl, 256:512]
            )

        # ---- emission schedule ----
        Ls = [None] * B_PER_CORE
        Ls[0] = emit_loads(0, first=True)
        Ls[1] = emit_loads(1)
        emit_prelim(0, Ls[0])
        emit_cast(0, Ls[0], 0)
        emit_spine(0, Ls[0], 0)
        emit_spine(0, Ls[0], 1)
        emit_ebp(0, Ls[0], 0)
        emit_gtail(0, Ls[0], 0)
        emit_cast(0, Ls[0], 1)
        emit_spine(0, Ls[0], 2)
        emit_ebp(0, Ls[0], 1)
        emit_gtail(0, Ls[0], 1)
        emit_spine(0, Ls[0], 3)
        emit_ebp(0, Ls[0], 2)
        emit_ebp(0, Ls[0], 3)
        emit_ebtail(0, Ls[0])
        emit_gtail(0, Ls[0], 2)
        emit_prelim(1, Ls[1])
        emit_cast(1, Ls[1], 0)
        emit_spine(1, Ls[1], 0)
        emit_gtail(0, Ls[0], 3)
        emit_spine(1, Ls[1], 1)
        emit_ebp(1, Ls[1], 0)
        emit_gtail(1, Ls[1], 0)
        for g in range(NG):
            emit_finish(0, Ls[0], g)
        emit_cast(1, Ls[1], 1)
        emit_spine(1, Ls[1], 2)
        emit_ebp(1, Ls[1], 1)
        emit_gtail(1, Ls[1], 1)
        emit_spine(1, Ls[1], 3)
        emit_ebp(1, Ls[1], 2)
        emit_ebp(1, Ls[1], 3)
        emit_ebtail(1, Ls[1])
        emit_finish(1, Ls[1], 0)
        emit_gtail(1, Ls[1], 2)
        emit_finish(1, Ls[1], 1)
        emit_gtail(1, Ls[1], 3)
        emit_finish(1, Ls[1], 2)
        emit_finish(1, Ls[1], 3)

    return nc


_NC_CACHE = None


def _get_nc():
    global _NC_CACHE
    if _NC_CACHE is None:
        _NC_CACHE = build_nc()
    return _NC_CACHE


def kernel(context, query, W, context_mask, query_mask):
    from concourse.bass_utils import run_bass_kernel_spmd

    context = np.ascontiguousarray(np.asarray(context, dtype=np.float32))
    query = np.ascontiguousarray(np.asarray(query, dtype=np.float32))
    W = np.ascontiguousarray(np.asarray(W, dtype=np.float32))
    context_mask = np.ascontiguousarray(np.asarray(context_mask, dtype=np.int32))
    query_mask = np.ascontiguousarray(np.asarray(query_mask, dtype=np.int32))

    nc = _get_nc()
    in_maps = []
    for c in range(N_CORES):
        sl = slice(c * B_PER_CORE, (c + 1) * B_PER_CORE)
        in_maps.append(
            {
                "context": context[sl],
                "query": query[sl],
                "W": W,
                "context_mask": context_mask[sl],
                "query_mask": query_mask[sl],
            }
        )
    res = run_bass_kernel_spmd(nc, in_maps, core_ids=list(range(N_CORES)))
    out = np.concatenate([res.results[c]["G"] for c in range(N_CORES)], axis=0)
    return out.astype(np.float32)


if __name__ == "__main__":
    from concourse.timeline_sim import TimelineSim

    nc = build_nc()
    dur = TimelineSim(nc).simulate()
    print(f"TimelineSim estimated duration: {dur:.0f} ns")


# revision 49
# speedup vs baseline: 1.0073x; 1.0073x over previous
"""BiDAF attention Trainium2 kernel (fp16-staged, DMA-roofline oriented).

Full-input contract: kernel(**inputs) takes the unsharded tensors
(context [16,2048,128], query [16,128,128], W [384],
context_mask [16,2048] i32, query_mask [16,128] i32) and returns
G = [16, 2048, 512] f32.

Sharding: data-parallel over batch across 8 NeuronCores (2 batches/core).
Each batch's math is fully local to a core, so no collectives.

Per-batch structure (16 c-tiles of 128 rows, in 4 groups of 4):
  per group (single fused pass):
    PE-transpose ctx (f32) -> ctxT fp16,
    S' = ctxT^T @ (qT*w_cq + w_c)   [s_c folded into the moving operand]
      + rank-1 (s_q + qmask) row    -> row-max m = max_q S' (q2c logits),
    T^T = qTwc^T @ ctxT, eT = exp(T^T + (s_q+qmask) bias),
    e_b = exp(m)*cmask, q2c/z partial accumulation,
    c2q = eT^T @ [q|1] -> divide by in-bank Z col (G2), G3 = ctx*c2q.
  per batch tail: q2c row * 1/z broadcast, then per group G4 = ctx*q2c
    and ONE fp16 store of the full 512-col slab.

The softmax over q is invariant to the per-c s_c term, so including it in
eT is exact; exp() is computed without max-subtraction (|S'| <= ~9, fp16
max 65504). All matmul operands are fp16 or f32-pairs; G is written to
DRAM as fp16 (halves store bytes; adds ~5e-4 rel err against a 2e-2
budget) and upcast to f32 on the host.

Scheduling notes (TimelineSim-verified):
- A PSUM tile may have consumers on only ONE engine: cross-engine readers
  of the same PSUM tile are serialized by the dependency tracker. The c2q
  normalize therefore divides by the in-bank Z column on a single engine
  per PSUM pair-bank (no shared reciprocal tile).
- PSUM pools need bufs=2 so group g+1's producers don't wait on group g's
  consumer (ping-pong serialization).
- Engine roles: Act = spine (ctxT copy, exp); Pool = bulk (G1 casts,
  normalize); DVE = row-max, G3/G4, small glue; PE = matmuls.
"""

import sys

sys.path.insert(0, "/opt/trn_rl_repo")

import numpy as np

import concourse.bass as bass
import concourse.tile as tile
from concourse import mybir
from concourse.masks import make_identity
from concourse.vector_clock import ScopedClock

B, C_LEN, Q_LEN, H = 16, 2048, 128, 128
N_CORES = 8
B_PER_CORE = B // N_CORES          # 2
NT = C_LEN // 128                  # 16 c-tiles per batch
NG = 4                             # groups per batch
GS = NT // NG                      # tiles per group (4)
F32 = mybir.dt.float32
F16 = mybir.dt.float16
I32 = mybir.dt.int32
AX = mybir.AxisListType.X
EXP = mybir.ActivationFunctionType.Exp

MAX_WAITS_PER_INST = 1


def _split_excess_waits(nc, insts):
    """Hoist all but one sync wait per instruction onto preceding nops.

    The walrus build in this container rejects >1 sync wait on an
    instruction's descriptor, while Tile's sem assignment freely attaches
    several. A nop on the same engine right before the instruction stalls
    the engine identically.
    """
    out = []
    for inst in insts:
        si = getattr(inst, "sync_info", None)
        waits = list(si.on_wait) if si is not None and si.on_wait else []
        if len(waits) > MAX_WAITS_PER_INST and type(inst).__name__.startswith("Inst"):
            keep = waits[:MAX_WAITS_PER_INST]
            extra = waits[MAX_WAITS_PER_INST:]
            for i in range(0, len(extra), MAX_WAITS_PER_INST):
                out.append(
                    mybir.InstNoOp(
                        name=nc.get_next_instruction_name(),
                        sync_info=mybir.SyncInfo(
                            on_wait=extra[i : i + MAX_WAITS_PER_INST], on_update=[]
                        ),
                        bass_nofuse=True,
                        engine=inst.engine,
                    )
                )
            inst.sync_info = mybir.SyncInfo(
                on_wait=keep, on_update=list(si.on_update or [])
            )
        out.append(inst)
    return out


class SplitDrainTileContext(tile.TileContext):
    """TileContext whose tail drain splits its sem waits across SP nops.

    The walrus build in this container rejects more than one sync wait on a
    TPB_CTRL instruction; the stock tail drain carries one wait per live proc.
    """

    def _lower_ordered_insts(self, ordered):
        for bb_name in list(ordered.keys()):
            ordered[bb_name] = _split_excess_waits(self.nc, ordered[bb_name])
        return super()._lower_ordered_insts(ordered)

    def _drain_and_barrier(self, tick_clock, wait_clock):
        nc = self.nc
        drain_inst = nc.sync.drain()
        wait_clock.add_sem_waits(
            drain_inst.ins, ScopedClock({None: tick_clock.global_clock})
        )
        si = drain_inst.ins.sync_info
        waits = list(si.on_wait) if si is not None and si.on_wait else []
        if waits:
            drain_inst.ins.sync_info = mybir.SyncInfo(
                on_wait=[], on_update=list(si.on_update or [])
            )
            # spread the tail waits across all engines; the all-engine
            # barrier below joins them, so they complete in parallel
            engs = [nc.sync, nc.vector, nc.scalar, nc.tensor, nc.gpsimd]
            for j, i in enumerate(range(0, len(waits), MAX_WAITS_PER_INST)):
                nop = engs[j % len(engs)].nop()
                nop.ins.sync_info = mybir.SyncInfo(
                    on_wait=waits[i : i + MAX_WAITS_PER_INST], on_update=[]
                )
        nc.all_engine_barrier()
        assert self.sems is not None
        popped = nc._tile_sem_poison_stack.pop()
        assert popped is self._sem_poison
        nc.clear_and_free_semaphores(list(self.sems.allocated().values()))
        nc.all_engine_barrier()


def build_nc() -> bass.Bass:
    nc = bass.Bass()
    ctx_d = nc.dram_tensor("context", [B_PER_CORE, C_LEN, H], F32, kind="ExternalInput")
    qry_d = nc.dram_tensor("query", [B_PER_CORE, Q_LEN, H], F32, kind="ExternalInput")
    w_d = nc.dram_tensor("W", [3 * H], F32, kind="ExternalInput")
    cm_d = nc.dram_tensor("context_mask", [B_PER_CORE, C_LEN], I32, kind="ExternalInput")
    qm_d = nc.dram_tensor("query_mask", [B_PER_CORE, Q_LEN], I32, kind="ExternalInput")
    g_d = nc.dram_tensor("G", [B_PER_CORE, C_LEN, 4 * H], F16, kind="ExternalOutput")

    from contextlib import ExitStack

    # psm bank region columns (f32): q2c partials 0:4, s_q 4:5,
    # z row [0:1,5:21], srowT [0:1,22:86]->f16[1,128],
    # q2crT [0:1,86:150]->f16[1,128], cmT 150:166
    QP0, SQ0, Z0, SRT0, QRT0, CMT0 = 0, 4, 5, 22, 86, 150

    with SplitDrainTileContext(nc) as tc, ExitStack() as es:
        consts = es.enter_context(tc.tile_pool(name="consts", bufs=1))
        bp = es.enter_context(tc.tile_pool(name="bp", bufs=2))
        pmm = es.enter_context(tc.tile_pool(name="pmm", bufs=2, space="PSUM"))
        pct = es.enter_context(tc.tile_pool(name="pct", bufs=2, space="PSUM"))
        pcq = es.enter_context(tc.tile_pool(name="pcq", bufs=2, space="PSUM"))
        psm = es.enter_context(tc.tile_pool(name="psm", bufs=2, space="PSUM"))

        ident16 = consts.tile([128, 128], F16)
        make_identity(nc, ident16)
        ident32 = consts.tile([128, 128], F32)
        make_identity(nc, ident32)
        ones_row16 = consts.tile([1, 128], F16)
        nc.vector.memset(ones_row16, 1.0)
        ones_col16 = consts.tile([128, 1], F16)
        nc.vector.memset(ones_col16, 1.0)
        w_all = consts.tile([128, 3], F32)
        w_q16 = consts.tile([128, 1], F16)

        def emit_loads(b, first=False):
            L = {}
            qry_f32 = bp.tile([128, 128], F32, tag="qry_f32")
            nc.sync.dma_start(out=qry_f32, in_=qry_d[b])
            if first:
                nc.sync.dma_start(out=w_all, in_=w_d.rearrange("(w h) -> h w", w=3))
                nc.vector.tensor_copy(out=w_q16, in_=w_all[:, 1:2])
            ctx_f32 = bp.tile([128, NT, 128], F32, tag="ctx_f32")
            ctx_view = ctx_d[b].rearrange("(t p) h -> p t h", p=128)
            nc.sync.dma_start(out=ctx_f32[:, 0:GS, :], in_=ctx_view[:, 0:GS, :])
            qm_i32 = bp.tile([128, 1], I32, tag="qm_i32")
            nc.sync.dma_start(out=qm_i32, in_=qm_d[b].rearrange("(q o) -> q o", o=1))
            for g in range(1, NG):
                sl = slice(g * GS, (g + 1) * GS)
                nc.sync.dma_start(out=ctx_f32[:, sl, :], in_=ctx_view[:, sl, :])
            cm_i32 = bp.tile([16, 128], I32, tag="cm_i32")
            nc.sync.dma_start(out=cm_i32, in_=cm_d[b].rearrange("(t p) -> t p", p=128))
            L.update(qry_f32=qry_f32, qm_i32=qm_i32, cm_i32=cm_i32, ctx_f32=ctx_f32)
            return L

        def emit_prelim(b, L):
            # query -> fp16 [q|1] moving operand; queryT; qTwc = qT*w_cq+w_c
            qrhs129 = bp.tile([128, 129], F16, tag="qrhs129")
            nc.vector.tensor_copy(out=qrhs129[:, 0:128], in_=L["qry_f32"])
            nc.gpsimd.memset(qrhs129[:, 128:129], 1.0)

            ps_qt = pct.tile([128, 128], F16, tag="ct")
            nc.tensor.transpose(ps_qt, qrhs129[:, 0:128], ident16)
            qT_sb = bp.tile([128, 128], F16, tag="qT_sb")
            nc.vector.tensor_copy(out=qT_sb, in_=ps_qt)
            qTwc = bp.tile([128, 128], F16, tag="qTwc")
            nc.vector.tensor_scalar(
                out=qTwc, in0=qT_sb, scalar1=w_all[:, 2:3], scalar2=w_all[:, 0:1],
                op0=mybir.AluOpType.mult, op1=mybir.AluOpType.add,
            )

            bs = psm.tile([128, 166], F32, tag="bs")
            # s_q column [q,1]
            nc.tensor.matmul(bs[:, SQ0 : SQ0 + 1], qT_sb, w_q16, start=True, stop=True)
            qm_f32 = bp.tile([128, 1], F32, tag="qm_f32")
            nc.vector.tensor_copy(out=qm_f32, in_=L["qm_i32"])
            nc.vector.tensor_scalar(
                out=qm_f32, in0=qm_f32, scalar1=1.0, scalar2=60000.0,
                op0=mybir.AluOpType.subtract, op1=mybir.AluOpType.mult,
            )
            sqm_col = bp.tile([128, 1], F32, tag="sqm_col")
            nc.vector.tensor_add(out=sqm_col, in0=bs[:, SQ0 : SQ0 + 1], in1=qm_f32)
            sqm16 = bp.tile([128, 1], F16, tag="sqm16")
            nc.vector.tensor_copy(out=sqm16, in_=sqm_col)
            nc.tensor.transpose(
                bs[0:1, SRT0 : SRT0 + 64].bitcast(F16), sqm16, ident16
            )
            # (s_q + qmask) row tiled 4x for the rank-1 accumulates
            srow512 = bp.tile([1, 512], F16, tag="srow512")
            srow_src = (
                bs[0:1, SRT0 : SRT0 + 64]
                .bitcast(F16)
                .rearrange("o (r q) -> o r q", r=1)
                .broadcast_to([1, GS, 128])
            )
            nc.vector.tensor_copy(
                out=srow512.rearrange("o (r q) -> o r q", q=128), in_=srow_src
            )

            # context mask: [16,128] i32 -> f32 -> PE transpose -> [c,16] fp16
            cm_f32 = bp.tile([16, 128], F32, tag="cm_f32")
            nc.gpsimd.tensor_copy(out=cm_f32, in_=L["cm_i32"])
            nc.tensor.transpose(
                bs[:, CMT0 : CMT0 + 16], cm_f32, ident32[0:16, 0:16]
            )
            cmf16 = bp.tile([128, 16], F16, tag="cmf16")
            nc.vector.tensor_copy(out=cmf16, in_=bs[:, CMT0 : CMT0 + 16])

            ctxT_buf = bp.tile([128, C_LEN], F16, tag="ctxT_buf")
            eT_buf = bp.tile([128, C_LEN], F16, tag="eT_buf")
            gbuf16 = bp.tile([128, NT, 4 * H], F16, tag="gbuf16")
            m_buf = bp.tile([128, NT], F32, tag="m_buf")
            e_raw = bp.tile([128, NT], F16, tag="e_raw")
            e_b = bp.tile([128, NT], F16, tag="e_b")
            g_view = g_d[b].rearrange("(t p) f -> p t f", p=128)
            L.update(
                qrhs129=qrhs129, qTwc=qTwc, sqm_col=sqm_col, srow512=srow512,
                cmf16=cmf16, ctxT_buf=ctxT_buf, eT_buf=eT_buf, gbuf16=gbuf16,
                m_buf=m_buf, e_raw=e_raw, e_b=e_b,
                g_view=g_view, bs=bs,
            )

        def emit_cast(b, L, half):
            # G1 = fp16(ctx), half a batch per op (fixed overheads amortize);
            # feeds only G3/G4/stores, all far from the spine.
            sl = slice(half * NT // 2, (half + 1) * NT // 2)
            nc.gpsimd.tensor_copy(
                out=L["gbuf16"][:, sl, 0:128], in_=L["ctx_f32"][:, sl, :]
            )

        def emit_spine(b, L, g):
            gsl = slice(g * GS, (g + 1) * GS)
            ctxT_buf, eT_buf = L["ctxT_buf"], L["eT_buf"]
            qTwc, srow512, bs = L["qTwc"], L["srow512"], L["bs"]
            c0 = g * GS * 128

            # ---- spine: transposes -> ctxT -> S'/max -> T^T -> exp ----
            ps_ct = pct.tile([128, 512], F32, tag="ct")
            for j in range(GS):
                t = g * GS + j
                nc.tensor.transpose(
                    ps_ct[:, j * 128 : (j + 1) * 128], L["ctx_f32"][:, t, :], ident32
                )
            nc.scalar.copy(out=ctxT_buf[:, c0 : c0 + 512], in_=ps_ct)

            ps_s = pmm.tile([128, GS, 128], F32, tag="mm")
            for j in range(GS):
                t = g * GS + j
                ctxT = ctxT_buf[:, t * 128 : (t + 1) * 128]
                nc.tensor.matmul(ps_s[:, j, :], ctxT, qTwc, start=True, stop=False)
                nc.tensor.matmul(
                    ps_s[:, j, :], ones_row16, srow512[:, j * 128 : (j + 1) * 128],
                    start=False, stop=True,
                )
            nc.vector.reduce_max(out=L["m_buf"][:, gsl], in_=ps_s, axis=AX)

            ps_tt = pmm.tile([128, 512], F32, tag="mm")
            nc.tensor.matmul(
                ps_tt, qTwc, ctxT_buf[:, c0 : c0 + 512], start=True, stop=True
            )
            nc.scalar.activation(
                out=eT_buf[:, c0 : c0 + 512], in_=ps_tt, func=EXP, bias=L["sqm_col"]
            )

        def emit_ebp(b, L, g):
            # q2c/z accumulation, once per batch (g == NG-1 sentinel)
            if g != NG - 1:
                return
            bs = L["bs"]
            nc.scalar.activation(out=L["e_raw"], in_=L["m_buf"], func=EXP)
            nc.vector.tensor_mul(out=L["e_b"], in0=L["e_raw"], in1=L["cmf16"])
            for t in range(NT):
                nc.tensor.matmul(
                    bs[:, QP0 : QP0 + 1],
                    L["gbuf16"][:, t, 0:128],
                    L["e_b"][:, t : t + 1],
                    start=(t == 0),
                    stop=(t == NT - 1),
                )
            nc.tensor.matmul(
                bs[0:1, Z0 : Z0 + NT], ones_col16, L["e_b"], start=True, stop=True
            )

        def emit_gtail(b, L, g):
            gsl = slice(g * GS, (g + 1) * GS)
            gbuf16, eT_buf, bs = L["gbuf16"], L["eT_buf"], L["bs"]
            # ---- c2q + normalize (G2) + G3, inline per group ----
            pcqs = []
            for j in range(2):
                t0 = g * GS + 2 * j
                ps_cq = pcq.tile([128, 258], F32, tag="cq")
                nc.tensor.matmul(
                    ps_cq[:, 0:129],
                    eT_buf[:, t0 * 128 : (t0 + 1) * 128],
                    L["qrhs129"],
                    start=True, stop=True,
                )
                nc.tensor.matmul(
                    ps_cq[:, 129:258],
                    eT_buf[:, (t0 + 1) * 128 : (t0 + 2) * 128],
                    L["qrhs129"],
                    start=True, stop=True,
                )
                pcqs.append(ps_cq)
            for j in range(2):
                t0 = g * GS + 2 * j
                ps_cq = pcqs[j]
                dr = bp.tile([128, 2], F32, tag=f"dr{t0 % 4}")
                nc.vector.reciprocal(out=dr, in_=ps_cq[:, 128:258:129])
                if b == 0 or j == 0:
                    num = ps_cq.rearrange("p (t c) -> p t c", t=2)[:, :, 0:128]
                    drb = (
                        dr.rearrange("p (t o) -> p t o", o=1)
                        .broadcast_to([128, 2, 128])
                    )
                    nc.vector.tensor_tensor(
                        out=gbuf16[:, t0 : t0 + 2, 128:256], in0=num, in1=drb,
                        op=mybir.AluOpType.mult,
                    )
                else:
                    nc.scalar.mul(
                        out=gbuf16[:, t0, 128:256], in_=ps_cq[:, 0:128],
                        mul=dr[:, 0:1],
                    )
                    nc.scalar.mul(
                        out=gbuf16[:, t0 + 1, 128:256], in_=ps_cq[:, 129:257],
                        mul=dr[:, 1:2],
                    )
            # store cols 0:256 now (512B contiguous rows, full DMA rate);
            # G3+G4 go out together in the batch tail, also at full rate.
            nc.sync.dma_start(
                out=L["g_view"][:, gsl, 0:256], in_=gbuf16[:, gsl, 0:256]
            )
            g3eng = nc.gpsimd if b == 0 else nc.vector
            g3eng.tensor_mul(
                out=gbuf16[:, gsl, 256:384],
                in0=gbuf16[:, gsl, 0:128],
                in1=gbuf16[:, gsl, 128:256],
            )

        def emit_ebtail(b, L):
            bs = L["bs"]
            q2c16 = bp.tile([128, 1], F16, tag="q2c16")
            nc.vector.tensor_copy(out=q2c16, in_=bs[:, QP0 : QP0 + 1])
            z_tot = bp.tile([1, 1], F32, tag="z_tot")
            nc.vector.reduce_sum(out=z_tot, in_=bs[0:1, Z0 : Z0 + NT], axis=AX)
            zr = bp.tile([1, 1], F32, tag="zr")
            nc.vector.reciprocal(out=zr, in_=z_tot)
            nc.tensor.transpose(
                bs[0:1, QRT0 : QRT0 + 64].bitcast(F16), q2c16, ident16
            )
            q2cr_sb = bp.tile([1, 128], F16, tag="q2cr_sb")
            nc.vector.tensor_scalar_mul(
                q2cr_sb, bs[0:1, QRT0 : QRT0 + 64].bitcast(F16), zr
            )
            ps_bc = pcq.tile([128, 128], F32, tag="cq")
            nc.tensor.matmul(ps_bc, ones_row16, q2cr_sb, start=True, stop=True)
            bc_sb = bp.tile([128, 128], F16, tag="bc_sb")
            nc.vector.tensor_copy(out=bc_sb, in_=ps_bc)
            L.update(bc_sb=bc_sb)

        def emit_finish(b, L, g):
            gsl = slice(g * GS, (g + 1) * GS)
            gbuf16 = L["gbuf16"]
            bc_bcast = (
                L["bc_sb"].rearrange("p (o q) -> p o q", o=1).broadcast_to([128, GS, 128])
            )
            eng = nc.gpsimd if b == 0 else nc.vector
            eng.tensor_mul(
                out=gbuf16[:, gsl, 384:512],
                in0=gbuf16[:, gsl, 0:128],
                in1=bc_bcast,
            )
            nc.sync.dma_start(
                out=L["g_view"][:, gsl, 256:512], in_=gbuf16[:, gsl, 256:512]
            )

        # ---- emission schedule ----
        Ls = [None] * B_PER_CORE
        Ls[0] = emit_loads(0, first=True)
        Ls[1] = emit_loads(1)
        emit_prelim(0, Ls[0])
        emit_cast(0, Ls[0], 0)
        emit_spine(0, Ls[0], 0)
        emit_spine(0, Ls[0], 1)
        emit_ebp(0, Ls[0], 0)
        emit_gtail(0, Ls[0], 0)
        emit_cast(0, Ls[0], 1)
        emit_spine(0, Ls[0], 2)
        emit_ebp(0, Ls[0], 1)
        emit_gtail(0, Ls[0], 1)
        emit_spine(0, Ls[0], 3)
        emit_ebp(0, Ls[0], 2)
        emit_ebp(0, Ls[0], 3)
        emit_ebtail(0, Ls[0])
        emit_gtail(0, Ls[0], 2)
        emit_prelim(1, Ls[1])
        emit_cast(1, Ls[1], 0)
        emit_spine(1, Ls[1], 0)
        emit_gtail(0, Ls[0], 3)
        emit_spine(1, Ls[1], 1)
        emit_ebp(1, Ls[1], 0)
        emit_gtail(1, Ls[1], 0)
        for g in range(NG):
            emit_finish(0, Ls[0], g)
        emit_cast(1, Ls[1], 1)
        emit_spine(1, Ls[1], 2)
        emit_ebp(1, Ls[1], 1)
        emit_gtail(1, Ls[1], 1)
        emit_spine(1, Ls[1], 3)
        emit_ebp(1, Ls[1], 2)
        emit_ebp(1, Ls[1], 3)
        emit_ebtail(1, Ls[1])
        emit_finish(1, Ls[1], 0)
        emit_gtail(1, Ls[1], 2)
        emit_finish(1, Ls[1], 1)
        emit_gtail(1, Ls[1], 3)
        emit_finish(1, Ls[1], 2)
        emit_finish(1, Ls[1], 3)

    return nc


_NC_CACHE = None


def _get_nc():
    global _NC_CACHE
    if _NC_CACHE is None:
        _NC_CACHE = build_nc()
    return _NC_CACHE


def kernel(context, query, W, context_mask, query_mask):
    from concourse.bass_utils import run_bass_kernel_spmd

    context = np.ascontiguousarray(np.asarray(context, dtype=np.float32))
    query = np.ascontiguousarray(np.asarray(query, dtype=np.float32))
    W = np.ascontiguousarray(np.asarray(W, dtype=np.float32))
    context_mask = np.ascontiguousarray(np.asarray(context_mask, dtype=np.int32))
    query_mask = np.ascontiguousarray(np.asarray(query_mask, dtype=np.int32))

    nc = _get_nc()
    in_maps = []
    for c in range(N_CORES):
        sl = slice(c * B_PER_CORE, (c + 1) * B_PER_CORE)
        in_maps.append(
            {
                "context": context[sl],
                "query": query[sl],
                "W": W,
                "context_mask": context_mask[sl],
                "query_mask": query_mask[sl],
            }
        )
    res = run_bass_kernel_spmd(nc, in_maps, core_ids=list(range(N_CORES)))
    out = np.concatenate([res.results[c]["G"] for c in range(N_CORES)], axis=0)
    return out.astype(np.float32)


if __name__ == "__main__":
    from concourse.timeline_sim import TimelineSim

    nc = build_nc()
    dur = TimelineSim(nc).simulate()
    print(f"TimelineSim estimated duration: {dur:.0f} ns")
